# revision 22
# baseline (speedup 1.0000x reference)
"""Trainium2 Bass kernel for nn_AGREE (group-member attention + predict MLP).

Data-parallel across 8 NeuronCores: B=16384 samples sharded 2048/core,
embedding tables + MLP weights replicated.

Per sample b:
  mem_e = user_table[member_ids[b]]            [50, 64]
  item_e = item_table[item_inputs[b]]          [64]
  h = relu(concat(mem_e, item_e) @ att_w1+b1)  [50, 16]
  scores = h @ att_w2 (+b2, softmax-invariant) [50]
  at_wt = softmax(scores masked to m <= member_lengths[b])
  g = at_wt @ mem_e + group_table[group_inputs[b]]
  y = sigmoid(relu([g*item, g, item] @ pred_w1 + pred_b1) @ pred_w2 + pred_b2)

Kernel layout strategy (per 128-sample tile):
  - one indirect DMA gathers all 50*128 member rows ([128, 50*64] SBUF)
  - PE transposes member-PAIRS ([128,128] -> [128,128] PSUM, K-stacked)
  - att MLP as matmuls with member-blocks stacked on K; scores via
    block-diagonal w2 matmul landing [128 samples, 50] directly
  - masked softmax on DVE/ACT; weighted member sum via fused
    scalar_tensor_tensor accumulation; predict MLP on PE.
"""

import sys

sys.path.insert(0, "/opt/trn_rl_repo")

import numpy as np

from concourse import bacc, bass, mybir
from concourse.tile import TileContext

NC = 8
B, M, D = 16384, 50, 64
BL = B // NC  # samples per core
P = 128
NT = BL // P  # sample tiles per core
HID = 16
G8 = 8  # members per score group
NGRP = (M + G8 - 1) // G8  # 7 (last group has 2 members)
F32 = mybir.dt.float32
I32 = mybir.dt.int32

NUM_USERS, NUM_ITEMS, NUM_GROUPS = 100000, 50000, 20000

AF = mybir.ActivationFunctionType
OP = mybir.AluOpType
AX = mybir.AxisListType

_CACHE = {}


def build_nc():
    # Bacc layer: its compile() legalizes sync waits (TRN2 allows at most one
    # wait per instruction; extra waits become event-semaphore carriers).
    nc = bacc.Bacc()

    # --- data inputs (per-core shards), host-arranged tile-major:
    # plane[p, t...] = value for sample t*128+p ---
    ids_ext = nc.declare_dram_parameter("m_ids", [P, NT * M], I32, isOutput=False)
    item_ext = nc.declare_dram_parameter("i_idx", [P, NT], I32, isOutput=False)
    grp_ext = nc.declare_dram_parameter("g_idx", [P, NT], I32, isOutput=False)
    len_ext = nc.declare_dram_parameter("m_len", [P, NT], F32, isOutput=False)
    user_ext = nc.declare_dram_parameter("user_t", [NUM_USERS, D], F32, isOutput=False)
    itab_ext = nc.declare_dram_parameter("item_t", [NUM_ITEMS, D], F32, isOutput=False)
    gtab_ext = nc.declare_dram_parameter("group_t", [NUM_GROUPS, D], F32, isOutput=False)

    # --- static (host-prepared weight rearrangements) ---
    ident_ext = nc.declare_dram_parameter("ident", [P, P], F32, isOutput=False)
    w1ib_ext = nc.declare_dram_parameter("w1i_b1", [D + 1, P], F32, isOutput=False)
    w1u2_ext = nc.declare_dram_parameter("w1u2", [P, 2 * HID], F32, isOutput=False)
    w2blk_ext = nc.declare_dram_parameter("w2blk", [P, G8], F32, isOutput=False)
    pw1a_ext = nc.declare_dram_parameter("pw1a", [2 * D, 8], F32, isOutput=False)
    pw1b_ext = nc.declare_dram_parameter("pw1b_b1", [D + 1, 8], F32, isOutput=False)
    pw2_ext = nc.declare_dram_parameter("pw2_b2", [9, 1], F32, isOutput=False)

    out_ext = nc.declare_dram_parameter("out", [BL, 1], F32, isOutput=True)

    with TileContext(nc) as tc:
        with (
            tc.tile_pool(name="const", bufs=1) as cn,
            tc.tile_pool(name="gall", bufs=2) as gp,
            tc.tile_pool(name="sbuf", bufs=3) as sb,
            tc.tile_pool(name="hts", bufs=2) as hb,
            tc.tile_pool(name="psA", bufs=1, space="PSUM") as psA,
            tc.tile_pool(name="psB", bufs=2, space="PSUM") as psB,
            tc.tile_pool(name="psC", bufs=1, space="PSUM") as psC,
            tc.tile_pool(name="psD", bufs=2, space="PSUM") as psD,
        ):
            # constants into SBUF once
            ident = cn.tile([P, P], F32)
            nc.sync.dma_start(out=ident[:], in_=ident_ext[:])
            w1ib = cn.tile([D + 1, P], F32)
            nc.sync.dma_start(out=w1ib[:], in_=w1ib_ext[:])
            w1u2 = cn.tile([P, 2 * HID], F32)
            nc.sync.dma_start(out=w1u2[:], in_=w1u2_ext[:])
            w2blk = cn.tile([P, G8], F32)
            nc.sync.dma_start(out=w2blk[:], in_=w2blk_ext[:])
            # device-generated member iota (avoids a DMA dep on the mask op,
            # whose TensorScalarPtr encoding has a single sync-wait slot)
            iota_i = cn.tile([P, M], I32)
            nc.gpsimd.iota(iota_i[:], pattern=[[1, M]], base=0, channel_multiplier=0)
            iota_m = cn.tile([P, M], F32)
            nc.vector.tensor_copy(out=iota_m[:], in_=iota_i[:])
            pw1a = cn.tile([2 * D, 8], F32)
            nc.sync.dma_start(out=pw1a[:], in_=pw1a_ext[:])
            pw1b = cn.tile([D + 1, 8], F32)
            nc.sync.dma_start(out=pw1b[:], in_=pw1b_ext[:])
            pw2 = cn.tile([9, 1], F32)
            nc.sync.dma_start(out=pw2[:], in_=pw2_ext[:])
            ids_all = cn.tile([P, NT * M], I32)
            nc.sync.dma_start(out=ids_all[:], in_=ids_ext[:])
            iidx_all = cn.tile([P, NT], I32)
            nc.sync.dma_start(out=iidx_all[:], in_=item_ext[:])
            gidx_all = cn.tile([P, NT], I32)
            nc.sync.dma_start(out=gidx_all[:], in_=grp_ext[:])
            len_all = cn.tile([P, NT], F32)
            nc.sync.dma_start(out=len_all[:], in_=len_ext[:])
            # absorb the len-plane DMA into the DVE clock once, so per-tile
            # TS-struct mask ops never carry a DMA wait themselves
            warm = cn.tile([P, 1], F32)
            nc.vector.tensor_copy(out=warm[:], in_=len_all[:, 0:1])

            for t in range(NT):
                r0 = t * P
                # ---- gathers (indices resident in SBUF) ----
                g_all = gp.tile([P, M * D], F32, tag="gall")
                nc.gpsimd.indirect_dma_start(
                    out=g_all[:],
                    out_offset=None,
                    in_=user_ext[:],
                    in_offset=bass.IndirectOffsetOnAxis(ap=ids_all[:, t * M : (t + 1) * M], axis=0),
                )
                i_rows = sb.tile([P, D], F32, tag="irows")
                nc.gpsimd.indirect_dma_start(
                    out=i_rows[:],
                    out_offset=None,
                    in_=itab_ext[:],
                    in_offset=bass.IndirectOffsetOnAxis(ap=iidx_all[:, t : t + 1], axis=0),
                )
                gr_rows = sb.tile([P, D], F32, tag="grrows")
                nc.gpsimd.indirect_dma_start(
                    out=gr_rows[:],
                    out_offset=None,
                    in_=gtab_ext[:],
                    in_offset=bass.IndirectOffsetOnAxis(ap=gidx_all[:, t : t + 1], axis=0),
                )

                # ---- item embedding transposed, with ones row for bias ----
                it_ps = psD.tile([D, P], F32, tag="tps")
                nc.tensor.transpose(out=it_ps[:], in_=i_rows[:], identity=ident[:])
                it65 = sb.tile([D + 1, P], F32, tag="it65")
                nc.vector.tensor_copy(out=it65[:D, :], in_=it_ps[:])
                nc.vector.memset(it65[D : D + 1, :], 1.0)

                # ---- attention MLP: scores [128 samples, 50] in PSUM ----
                sc_ps = psC.tile([P, M], F32, tag="scps")
                for g in range(NGRP):
                    mg = min(G8, M - g * G8)  # members in this group
                    rows = mg * HID
                    # two [64, P] psum halves: matmul outs may only start at
                    # base partition 0/32/64, so partition 96 is unreachable
                    # within one [128, P] tile.
                    rows_a = min(rows, D)
                    rows_b = rows - rows_a
                    hp_a = psA.tile([D, 512], F32, tag="hpa")
                    hp_b = None
                    if rows_b > 0:
                        hp_b = psA.tile([D, 512], F32, tag="hpb")
                    # member pairs open each 32-row region (start=True);
                    # the item part (+b1) then accumulates over the union and
                    # closes the chain. Region-consistent with the sim's
                    # zero-region bookkeeping; equivalent on HW.
                    for q in range((mg + 1) // 2):
                        m0 = g * G8 + 2 * q
                        pair_ps = psB.tile([P, P], F32, tag="pair")
                        nc.tensor.transpose(
                            out=pair_ps[:],
                            in_=g_all[:, m0 * D : (m0 + 2) * D],
                            identity=ident[:],
                        )
                        pair_sb = sb.tile([P, P], F32, tag="pairsb")
                        nc.vector.tensor_copy(out=pair_sb[:], in_=pair_ps[:])
                        hp_t = hp_a if q < 2 else hp_b
                        off = 2 * HID * (q % 2)
                        nc.tensor.matmul(
                            out=hp_t[off : off + 2 * HID, :P],
                            lhsT=w1u2[:],
                            rhs=pair_sb[:],
                            start=True,
                            stop=False,
                            skip_group_check=True,
                        )
                    nc.tensor.matmul(
                        out=hp_a[:rows_a, :P], lhsT=w1ib[:, :rows_a], rhs=it65[:],
                        start=False, stop=True, skip_group_check=True,
                    )
                    if rows_b > 0:
                        nc.tensor.matmul(
                            out=hp_b[:rows_b, :P], lhsT=w1ib[:, :rows_b], rhs=it65[:],
                            start=False, stop=True, skip_group_check=True,
                        )
                    # relu -> SBUF
                    ht = hb.tile([P, P], F32, tag="ht")
                    nc.any.tensor_scalar_max(
                        out=ht[:rows_a, :], in0=hp_a[:rows_a, :P], scalar1=0.0
                    )
                    if rows_b > 0:
                        nc.any.tensor_scalar_max(
                            out=ht[D : D + rows_b, :],
                            in0=hp_b[:rows_b, :P],
                            scalar1=0.0,
                        )
                    # scores for this group land as [128 samples, mg]
                    nc.tensor.matmul(
                        out=sc_ps[:, g * G8 : g * G8 + mg],
                        lhsT=ht[:rows, :],
                        rhs=w2blk[:rows, :mg],
                        start=True,
                        stop=True,
                    )

                # ---- masked softmax over members ----
                msk = sb.tile([P, M], F32, tag="msk")
                nc.vector.tensor_scalar(
                    out=msk[:], in0=iota_m[:], scalar1=len_all[:, t : t + 1],
                    scalar2=None, op0=OP.is_le,
                )
                sc_raw = sb.tile([P, M], F32, tag="scraw")
                nc.vector.tensor_copy(out=sc_raw[:], in_=sc_ps[:])
                sc_sb = sb.tile([P, M], F32, tag="scsb")
                # (scores + 30) * mask  (shift keeps real scores > 0; masked -> 0)
                nc.vector.scalar_tensor_tensor(
                    out=sc_sb[:], in0=sc_raw[:], scalar=30.0, in1=msk[:],
                    op0=OP.add, op1=OP.mult,
                )
                mx = sb.tile([P, 1], F32, tag="mx")
                nc.vector.tensor_reduce(out=mx[:], in_=sc_sb[:], axis=AX.X, op=OP.max)
                negmx = sb.tile([P, 1], F32, tag="negmx")
                nc.scalar.activation(
                    out=negmx[:], in_=mx[:], func=AF.Copy, scale=-1.0
                )
                e_sb = sb.tile([P, M], F32, tag="esb")
                nc.scalar.activation(
                    out=e_sb[:], in_=sc_sb[:], func=AF.Exp, bias=negmx[:], scale=1.0
                )
                z_sb = sb.tile([P, 1], F32, tag="zsb")
                nc.vector.tensor_reduce(out=z_sb[:], in_=e_sb[:], axis=AX.X, op=OP.add)
                rz = sb.tile([P, 1], F32, tag="rz")
                nc.vector.reciprocal(out=rz[:], in_=z_sb[:])

                # ---- weighted member sum (unnormalized), fused MAC chain ----
                e_dve = sb.tile([P, M], F32, tag="edve")
                nc.vector.tensor_copy(out=e_dve[:], in_=e_sb[:])
                probe = sb.tile([P, 2], F32, tag="probe")
                nc.vector.tensor_copy(out=probe[:, 0:1], in_=g_all[:, 0:1])
                nc.vector.tensor_copy(out=probe[:, 1:2], in_=gr_rows[:, 0:1])
                accA = sb.tile([P, D], F32, tag="accA")
                accB = sb.tile([P, D], F32, tag="accB")
                nc.vector.tensor_scalar_mul(
                    out=accA[:], in0=g_all[:, :D], scalar1=e_dve[:, 0:1]
                )
                cur, nxt = accA, accB
                for m in range(1, M):
                    nc.vector.scalar_tensor_tensor(
                        out=nxt[:], in0=g_all[:, m * D : (m + 1) * D],
                        scalar=e_dve[:, m : m + 1], in1=cur[:],
                        op0=OP.mult, op1=OP.add,
                    )
                    cur, nxt = nxt, cur
                # g = acc * (1/Z) + group_e
                g_sb = sb.tile([P, D], F32, tag="gsb")
                nc.vector.scalar_tensor_tensor(
                    out=g_sb[:], in0=cur[:], scalar=rz[:], in1=gr_rows[:],
                    op0=OP.mult, op1=OP.add,
                )

                # ---- predict MLP ----
                gt_ps = psD.tile([D, P], F32, tag="tps")
                nc.tensor.transpose(out=gt_ps[:], in_=g_sb[:], identity=ident[:])
                # gstack rows: [gT (0:64) ; elemT = gT*itemT (64:128)] — both
                # tensor_tensor inputs must share a base partition, so the
                # product writes the upper half. pw1a rows are ordered to match.
                gstack = sb.tile([2 * D, P], F32, tag="gstack")
                nc.vector.tensor_copy(out=gstack[:D, :], in_=gt_ps[:])
                nc.vector.tensor_tensor(
                    out=gstack[D : 2 * D, :], in0=gstack[:D, :], in1=it65[:D, :],
                    op=OP.mult,
                )
                pp = psC.tile([8, 512], F32, tag="pp")
                nc.tensor.matmul(
                    out=pp[:, :P], lhsT=pw1a[:], rhs=gstack[:], start=True, stop=False
                )
                nc.tensor.matmul(
                    out=pp[:, :P], lhsT=pw1b[:], rhs=it65[:], start=False, stop=True
                )
                ph = sb.tile([9, P], F32, tag="ph")
                nc.vector.memset(ph[:], 1.0)
                nc.vector.tensor_scalar_max(out=ph[:8, :], in0=pp[:, :P], scalar1=0.0)
                y_ps = psD.tile([1, P], F32, tag="tps")
                nc.tensor.matmul(
                    out=y_ps[:], lhsT=pw2[:], rhs=ph[:], start=True, stop=True
                )
                y_sb = sb.tile([1, P], F32, tag="ysb")
                nc.scalar.activation(out=y_sb[:], in_=y_ps[:], func=AF.Sigmoid)
                nc.sync.dma_start(out=out_ext[r0 : r0 + P, :], in_=y_sb[:])

    nc.compile()
    return nc


def _statics(att_w1, att_b1, att_w2, pred_w1, pred_b1, pred_w2, pred_b2):
    f = np.float32
    ident = np.eye(P, dtype=f)
    # item-part weights + bias row, replicated across the 8 member blocks
    w1i_b1 = np.zeros((D + 1, P), dtype=f)
    for j in range(G8):
        w1i_b1[:D, j * HID : (j + 1) * HID] = att_w1[D:, :]
        w1i_b1[D, j * HID : (j + 1) * HID] = att_b1
    # member-pair projection: blockdiag(W1u, W1u)
    w1u2 = np.zeros((P, 2 * HID), dtype=f)
    w1u2[:D, :HID] = att_w1[:D, :]
    w1u2[D:, HID:] = att_w1[:D, :]
    # block-diagonal w2 for scores
    w2blk = np.zeros((P, G8), dtype=f)
    for j in range(G8):
        w2blk[j * HID : (j + 1) * HID, j] = att_w2[:, 0]
    # rows: [g-part (pred_w1[64:128]) ; elem-part (pred_w1[0:64])]
    pw1a = np.concatenate([pred_w1[D : 2 * D, :], pred_w1[:D, :]], axis=0).astype(f)
    pw1b_b1 = np.concatenate([pred_w1[2 * D :, :], pred_b1[None, :]], axis=0).astype(f)
    pw2_b2 = np.concatenate([pred_w2, pred_b2[None, :]], axis=0).astype(f)
    return dict(
        ident=ident, w1i_b1=w1i_b1, w1u2=w1u2, w2blk=w2blk,
        pw1a=pw1a, pw1b_b1=pw1b_b1, pw2_b2=pw2_b2,
    )


def make_in_maps(**inputs):
    st = _statics(
        np.asarray(inputs["att_w1"], np.float32),
        np.asarray(inputs["att_b1"], np.float32),
        np.asarray(inputs["att_w2"], np.float32),
        np.asarray(inputs["pred_w1"], np.float32),
        np.asarray(inputs["pred_b1"], np.float32),
        np.asarray(inputs["pred_w2"], np.float32),
        np.asarray(inputs["pred_b2"], np.float32),
    )
    def tile_major(x):
        # [BL(, k)] -> [P, NT(*k)]: column-block t holds samples t*128..t*128+127
        x = x.reshape(NT, P, -1)
        return np.ascontiguousarray(x.transpose(1, 0, 2).reshape(P, -1))

    m_ids = np.asarray(inputs["member_ids"], np.int32).reshape(NC, BL, M)
    i_idx = np.asarray(inputs["item_inputs"], np.int32).reshape(NC, BL)
    g_idx = np.asarray(inputs["group_inputs"], np.int32).reshape(NC, BL)
    m_len = np.asarray(inputs["member_lengths"], np.float32).reshape(NC, BL)
    user_t = np.ascontiguousarray(np.asarray(inputs["user_table"], np.float32))
    item_t = np.ascontiguousarray(np.asarray(inputs["item_table"], np.float32))
    group_t = np.ascontiguousarray(np.asarray(inputs["group_table"], np.float32))

    in_maps = []
    for c in range(NC):
        in_maps.append(
            {
                "m_ids": tile_major(m_ids[c]),
                "i_idx": tile_major(i_idx[c]),
                "g_idx": tile_major(g_idx[c]),
                "m_len": tile_major(m_len[c]),
                "user_t": user_t,
                "item_t": item_t,
                "group_t": group_t,
                "ident": st["ident"],
                "w1i_b1": st["w1i_b1"],
                "w1u2": st["w1u2"],
                "w2blk": st["w2blk"],
                "pw1a": st["pw1a"],
                "pw1b_b1": st["pw1b_b1"],
                "pw2_b2": st["pw2_b2"],
            }
        )
    return in_maps


def get_nc():
    if "nc" not in _CACHE:
        _CACHE["nc"] = build_nc()
    return _CACHE["nc"]


def kernel(**inputs):
    from concourse.bass_utils import run_bass_kernel_spmd

    nc = get_nc()
    in_maps = make_in_maps(**inputs)
    res = run_bass_kernel_spmd(nc, in_maps, core_ids=list(range(NC)))
    return np.concatenate([r["out"] for r in res.results], axis=0)
